# revision 1
# baseline (speedup 1.0000x reference)
"""Single-head attention layer (B=8, S=2048, F=D=512) on 8 Trainium2 cores.

Sharding: data-parallel over batch — core b computes batch element b entirely
on-chip (x[b] is 4 MB, weights 3 MB; everything fits in SBUF).

Per-core plan (all matmuls contract along the SBUF partition dim):
  1. x^T:    load x [S,F] tiles, add the (host-precomputed, constant)
             sinusoidal positional encoding, PE-transpose into x^T [F,S].
  2. QKV:    Q^T[d,s] = Wq[f,d]^T-matmul: lhsT=Wq tile, rhs=x^T  -> [D,S]
             K^T      likewise; V[s,d]: lhsT=x^T s-block, rhs=Wv -> [S,D]
             (V gets a 513th column of ones for the softmax denominator.)
  3. scores: S^T[j,i] tiles [128j, 512i]: lhsT=K^T j-block, rhs=Q^T i-chunk.
             exp((QK)/sqrt(D)) on ScalarE straight out of PSUM -> P^T tiles.
             No max-subtraction: scores are O(1) gaussians, exp stays finite,
             and softmax is shift-invariant so the result matches.
  4. out:    out[i,d] = P^T-as-lhsT @ V, accumulated over j in PSUM; the ones
             column of V lands the softmax row-sum in psum column 256 of the
             second half.  Normalize with DVE reciprocal + tensor_scalar_mul.
"""

import math
import os

import numpy as np

import bass_rust
import concourse.bass as bass
import concourse.tile as tile
from concourse import mybir
from concourse.bass_utils import run_bass_kernel_spmd

B, S, F, D = 8, 2048, 512, 512
P = 128
CH = 512  # i-chunk width for the score phase

# matmul input dtype: "bfloat16" | "float32r" | "float32"
MM_DT_NAME = os.environ.get("ATTN_MM_DT", "float32r")

# scheduling variant knobs (HW A/B testing): "pe_via:tpose_pool:tile0_first"
_v = os.environ.get("ATTN_VARIANT", "accum:psA:0:0").split(":")
VARIANT = {
    "pe_via": _v[0],
    "tpose_pool": _v[1],
    "tile0_first": bool(int(_v[2])),
    # one 1MB DMA per 512-row s-chunk of x instead of four 256KB DMAs
    "xchunk": bool(int(_v[3])) if len(_v) > 3 else False,
}

_WAIT_LIMIT = 1  # this walrus build allows one sync-wait command per inst


def _split_waits(nc, limit=_WAIT_LIMIT):
    """Move excess sync-waits onto NoOps inserted before the instruction.

    Waits execute on the engine sequencer before dispatch, so a chain of
    single-wait NOPs followed by the original instruction is equivalent.
    """
    cnt = 0
    for fn in nc.m.functions:
        for bb in fn.blocks:
            new_insts = []
            for ins in bb.instructions:
                si = ins.sync_info
                if si is not None and si.on_wait and len(si.on_wait) > limit:
                    waits = list(si.on_wait)
                    head, tail = waits[:-limit], waits[-limit:]
                    for i in range(0, len(head), limit):
                        nop = mybir.InstNoOp(
                            name=f"{ins.name}-wsplit{cnt}", ins=[], outs=[]
                        )
                        cnt += 1
                        nop.engine = ins.engine
                        nop.sync_info = bass_rust.SyncInfo(
                            on_wait=head[i : i + limit], on_update=[]
                        )
                        new_insts.append(nop)
                    ins.sync_info = bass_rust.SyncInfo(
                        on_wait=tail, on_update=list(si.on_update or [])
                    )
                new_insts.append(ins)
            bb.instructions[:] = new_insts
    return cnt


def _pe_table():
    """Sinusoidal positional encoding [S, F], float32, matching the standard
    transformer formula computed in float32."""
    pos = np.arange(S, dtype=np.float32)[:, None]
    i = np.arange(F)[None, :]
    angle = pos / np.power(
        np.float32(10000.0), (2 * (i // 2)).astype(np.float32) / F
    ).astype(np.float32)
    return np.where(i % 2 == 0, np.sin(angle), np.cos(angle)).astype(np.float32)


def _build(mm_dt_name, reps=1):
    # reps>1 repeats the whole body inside one NEFF (same tiles/tags, so no
    # extra SBUF) — used by the timing harness to difference out dispatch
    # and one-time preamble/tail costs.
    f32 = mybir.dt.float32
    # float32r is a real storage dtype: the BIR verifier requires every tensor
    # consumed by an fp32r matmul to have been *written* as float32r (compute
    # ops round on write; plain DMA does not). So matmul operands are stored
    # in store_dt and produced by copy/activation ops, never raw DMA.
    store_dt = getattr(mybir.dt, mm_dt_name)

    def mmv(ap):
        return ap

    nc = bass.Bass()
    x = nc.dram_tensor("x", [S, F], f32, kind="ExternalInput")
    wq = nc.dram_tensor("wq", [F, D], f32, kind="ExternalInput")
    wk = nc.dram_tensor("wk", [F, D], f32, kind="ExternalInput")
    wv = nc.dram_tensor("wv", [F, D], f32, kind="ExternalInput")
    out = nc.dram_tensor("out", [S, D], f32, kind="ExternalOutput")
    pe_d = nc.inline_tensor(_pe_table(), "pe")
    ident_d = nc.inline_tensor(np.eye(P, dtype=np.float32), "ident")

    nF, nS, nD = F // P, S // P, D // P
    nIC = S // CH
    scale = 1.0 / math.sqrt(D)
    Exp = mybir.ActivationFunctionType.Exp

    from contextlib import ExitStack

    pe_via = VARIANT.get("pe_via", "accum")
    tpose_in_psO = VARIANT.get("tpose_pool", "psO") == "psO"
    tile0_first = VARIANT.get("tile0_first", False)
    xchunk = VARIANT.get("xchunk", False)

    with tile.TileContext(nc) as tc, ExitStack() as ctx:
        const = ctx.enter_context(tc.tile_pool(name="const", bufs=1))
        persist = ctx.enter_context(tc.tile_pool(name="persist", bufs=1))
        if xchunk:
            xin = ctx.enter_context(tc.tile_pool(name="xin", bufs=3))
            ppool = ctx.enter_context(tc.tile_pool(name="ppool", bufs=16))
        elif pe_via == "dve":
            xin = ctx.enter_context(tc.tile_pool(name="xin", bufs=4))
            pein = ctx.enter_context(tc.tile_pool(name="pein", bufs=2))
            ppool = ctx.enter_context(tc.tile_pool(name="ppool", bufs=16))
        else:
            xin = ctx.enter_context(tc.tile_pool(name="xin", bufs=6))
            ppool = ctx.enter_context(tc.tile_pool(name="ppool", bufs=17))
        wstg = ctx.enter_context(tc.tile_pool(name="wstg", bufs=2))
        opool = ctx.enter_context(tc.tile_pool(name="opool", bufs=2))
        rpool = ctx.enter_context(tc.tile_pool(name="rpool", bufs=2))
        psA = ctx.enter_context(tc.tile_pool(name="psA", bufs=4, space="PSUM"))
        psO = ctx.enter_context(tc.tile_pool(name="psO", bufs=4, space="PSUM"))
        psT = psO if tpose_in_psO else psA

        ident = const.tile([P, P], f32, tag="ident", name="ident")
        nc.sync.dma_start(ident, ident_d[:, :])
        ones = const.tile([P, 2], f32, tag="ones", name="ones")
        nc.vector.memset(ones, 1.0)

        for _rep in range(reps):
            def load_x_tile(si):
                xt = xin.tile([P, F], f32, tag="xin", name="xin")
                nc.sync.dma_start(xt, x[si * P : (si + 1) * P, :])
                if pe_via == "dve":
                    pt = pein.tile([P, F], f32, tag="pein", name="pein")
                    nc.sync.dma_start(pt, pe_d[si * P : (si + 1) * P, :])
                    nc.vector.tensor_add(xt, xt, pt)
                else:
                    # pe added by the DMA engine (SWDGE accumulate-add)
                    nc.gpsimd.dma_start(
                        xt,
                        pe_d[si * P : (si + 1) * P, :],
                        accum_op=mybir.AluOpType.add,
                    )
                # all 4 transposes write quarters of one PSUM bank, then a
                # single strided copy evacuates them (4x fewer DVE/ACT ops)
                tg = "psO" if tpose_in_psO else "psA"
                pst = psT.tile([P, nF * P], f32, tag=tg, name="psT")
                for kf in range(nF):
                    nc.tensor.transpose(
                        pst[:, kf * P : (kf + 1) * P],
                        xt[:, kf * P : (kf + 1) * P],
                        ident,
                    )
                nc.any.tensor_copy(
                    xTall[:, :, si * P : (si + 1) * P],
                    pst.rearrange("p (k s) -> p k s", k=nF),
                )

            def load_x_half(h):
                # one 512KB DMA for 256 rows (2 s-tiles) of x:
                # dst [128, 2, 512], row (2h+t)*128+p lands at [p, t, :]
                xc = xin.tile([P, 2, F], f32, tag="xin", name="xin")
                src_v = x[h * 2 * P : (h + 1) * 2 * P, :].rearrange(
                    "(t p) f -> p t f", p=P
                )
                nc.sync.dma_start(xc, src_v)
                for t in range(2):
                    si = 2 * h + t
                    nc.gpsimd.dma_start(
                        xc[:, t, :],
                        pe_d[si * P : (si + 1) * P, :],
                        accum_op=mybir.AluOpType.add,
                    )
                    for kf in range(nF):
                        tg = "psO" if tpose_in_psO else "psA"
                        pst = psT.tile([P, P], f32, tag=tg, name="psT")
                        nc.tensor.transpose(
                            pst, xc[:, t, kf * P : (kf + 1) * P], ident
                        )
                        nc.any.tensor_copy(
                            xT[kf][:, si * P : (si + 1) * P], pst
                        )

            def load_x_chunk(c):
                load_x_half(2 * c)
                load_x_half(2 * c + 1)

            xTall = persist.tile(
                [P, nF, S], store_dt, tag="xTall", name="xTall"
            )
            xT = [xTall[:, k, :] for k in range(nF)]
            if xchunk:
                # first chunk before the weight DMAs so PE starts early
                load_x_chunk(0)
            elif tile0_first:
                # x tile 0 before the weight DMAs so PE transposes start early
                load_x_tile(0)

            # ---- weights into SBUF (cast to store_dt if needed) ----
            wsb = {}
            for nm, w in (("q", wq), ("k", wk), ("v", wv)):
                for kf in range(nF):
                    t = persist.tile(
                        [P, D], store_dt, tag=f"w{nm}{kf}", name=f"w{nm}{kf}"
                    )
                    if store_dt == f32:
                        nc.sync.dma_start(t, w[kf * P : (kf + 1) * P, :])
                    else:
                        st = wstg.tile([P, D], f32, tag="wstg", name="wstg")
                        nc.sync.dma_start(st, w[kf * P : (kf + 1) * P, :])
                        nc.any.tensor_copy(t, st)  # rounds f32 -> store_dt
                    wsb[(nm, kf)] = t

            # ---- x + pe -> x^T, interleaved with QKV per 512-col s-chunk so
            # PE matmuls overlap the input DMA latency ----
            QT = [
                persist.tile([P, S], store_dt, tag=f"QT{m}", name=f"QT{m}")
                for m in range(nD)
            ]
            KT = [
                persist.tile([P, S], store_dt, tag=f"KT{m}", name=f"KT{m}")
                for m in range(nD)
            ]
            V = [
                persist.tile([P, 520], store_dt, tag=f"V{si}", name=f"V{si}")
                for si in range(nS)
            ]
            for c in range(S // 512):
                if xchunk:
                    if c > 0:
                        load_x_chunk(c)
                else:
                    for t in range(512 // P):
                        si = (512 // P) * c + t
                        if not (tile0_first and si == 0):
                            load_x_tile(si)
                # Q^T, K^T columns for this s-chunk
                for dst, nm in ((QT, "q"), (KT, "k")):
                    for m in range(nD):
                        ps = psA.tile([P, 512], f32, tag="psA", name="psQK")
                        for kf in range(nF):
                            nc.tensor.matmul(
                                ps,
                                mmv(wsb[(nm, kf)][:, m * P : (m + 1) * P]),
                                mmv(xT[kf][:, c * 512 : (c + 1) * 512]),
                                start=(kf == 0),
                                stop=(kf == nF - 1),
                            )
                        nc.any.tensor_copy(dst[m][:, c * 512 : (c + 1) * 512], ps)
                # V rows for this s-chunk (two ones columns at 512/513 keep the
                # rowsum matmul free size (258) even, as fp32r requires)
                for t in range(512 // P):
                    si = (512 // P) * c + t
                    ps = psA.tile([P, 512], f32, tag="psA", name="psV")
                    for kf in range(nF):
                        nc.tensor.matmul(
                            ps,
                            mmv(xT[kf][:, si * P : (si + 1) * P]),
                            mmv(wsb[("v", kf)]),
                            start=(kf == 0),
                            stop=(kf == nF - 1),
                        )
                    nc.any.tensor_copy(V[si][:, 0:D], ps)
                    nc.vector.tensor_copy(V[si][:, D : D + 2], ones)

            # ---- attention, one i-chunk (512 queries) at a time ----
            for ic in range(nIC):
                Ptiles = []
                for j in range(nS):
                    ps = psA.tile([P, CH], f32, tag="psA", name="psS")
                    for kd in range(nD):
                        nc.tensor.matmul(
                            ps,
                            mmv(KT[kd][:, j * P : (j + 1) * P]),
                            mmv(QT[kd][:, ic * CH : (ic + 1) * CH]),
                            start=(kd == 0),
                            stop=(kd == nD - 1),
                        )
                    Pj = ppool.tile([P, CH], store_dt, tag="ppool", name="Pj")
                    nc.scalar.activation(Pj, ps, Exp, scale=scale)
                    Ptiles.append(Pj)
                for ib in range(CH // P):
                    i0 = ic * CH + ib * P
                    pa = psO.tile([P, 256], f32, tag="psO", name="pa")
                    pb = psO.tile([P, 258], f32, tag="psO", name="pb")
                    for j in range(nS):
                        lhsT = mmv(Ptiles[j][:, ib * P : (ib + 1) * P])
                        nc.tensor.matmul(
                            pa,
                            lhsT,
                            mmv(V[j][:, 0:256]),
                            start=(j == 0),
                            stop=(j == nS - 1),
                        )
                        nc.tensor.matmul(
                            pb,
                            lhsT,
                            mmv(V[j][:, 256:514]),
                            start=(j == 0),
                            stop=(j == nS - 1),
                        )
                    rec = rpool.tile([P, 1], f32, tag="rpool", name="rec")
                    nc.vector.reciprocal(rec, pb[:, 256:257])
                    ot = opool.tile([P, D], f32, tag="opool", name="ot")
                    nc.vector.tensor_scalar_mul(ot[:, 0:256], pa, rec)
                    nc.scalar.mul(ot[:, 256:512], pb[:, 0:256], rec)
                    nc.sync.dma_start(out[i0 : i0 + P, :], ot)

    _split_waits(nc)
    return nc


_built = None


def _get_built():
    global _built
    if _built is None:
        _built = _build(MM_DT_NAME)
    return _built


def kernel(x, Wq, Wk, Wv):
    nc = _get_built()
    x = np.asarray(x, dtype=np.float32)
    Wq = np.asarray(Wq, dtype=np.float32)
    Wk = np.asarray(Wk, dtype=np.float32)
    Wv = np.asarray(Wv, dtype=np.float32)
    in_maps = [
        {"x": np.ascontiguousarray(x[b]), "wq": Wq, "wk": Wk, "wv": Wv}
        for b in range(B)
    ]
    try:
        res = run_bass_kernel_spmd(nc, in_maps, list(range(B)))
    except Exception:
        # transient device wedge: ask NRT to reset cores and retry once
        os.environ["NEURON_RT_RESET_CORES"] = "1"
        res = run_bass_kernel_spmd(nc, in_maps, list(range(B)))
    return np.stack([res.results[b]["out"] for b in range(B)], axis=0)

